# revision 1
# baseline (speedup 1.0000x reference)
"""Distributed Trainium2 Bass kernel for AGFMAttentionDPLM2.

Sharding: 8 cores = 2 (batch) x 4 (head groups of 4 heads).
Per core: LayerNorm (modulate folded into effective QKV weights host-side),
QKV projections in transposed layout, RoPE via permutation-matmul, attention
in scoresT[k,q] orientation (softmax denominator from a ones-column in v),
AllGather of ctxT over the 4-core batch group, column-sharded output
projection with gate/residual epilogue.
"""
import sys

for _p in ("/opt/trn_rl_repo", "/opt/pypackages"):
    if _p not in sys.path:
        sys.path.append(_p)

import numpy as np
import ml_dtypes

import concourse.bass as bass
import concourse.mybir as mybir
import concourse.tile as tile
from concourse import bacc
from concourse.bass_utils import run_bass_kernel_spmd

B, S, H, NH, HD = 2, 2048, 1024, 16, 64
EPS = 1e-5
NCORES = 8
GPC = 4            # cores per batch group (head-tensor-parallel degree)
HPG = NH // GPC    # heads per core = 4
OC = HPG * HD      # feature cols per core = 256
HALF = S // 2

F32 = mybir.dt.float32
BF16 = mybir.dt.bfloat16

_CACHE = {}


def _rope_tables():
    inv_freq = 1.0 / (10000.0 ** (np.arange(0, HD, 2, dtype=np.float64) / HD))
    t = np.arange(HALF, dtype=np.float64)
    freqs = np.outer(t, inv_freq)                       # [1024, 32]
    emb = np.concatenate([freqs, freqs], axis=-1)       # [1024, 64]
    cos = np.cos(emb).T                                 # [64, 2048->]
    sin = np.sin(emb).T
    cos_full = np.concatenate([cos, cos], axis=1)       # [64, 2048]
    sin_full = np.concatenate([sin, sin], axis=1)
    cos2 = np.concatenate([cos_full, cos_full], axis=0)  # [128, 2048] pair-stacked
    sin2 = np.concatenate([sin_full, sin_full], axis=0)
    return cos2.astype(ml_dtypes.bfloat16), sin2.astype(ml_dtypes.bfloat16)


def _perm_matrix():
    # lhsT for rot = perm.T @ q : rot[d] = -q[d+32] (d%64<32), +q[d-32] otherwise
    p = np.zeros((128, 128), np.float32)
    for blk in range(2):
        o = blk * 64
        for d in range(32):
            p[o + 32 + d, o + d] = -1.0
            p[o + d, o + 32 + d] = 1.0
    return p.astype(ml_dtypes.bfloat16)


def _build_core_inputs(inputs):
    """Host-side shard + fold. Returns list of 8 dicts of numpy arrays."""
    f32 = np.float32
    hs = np.asarray(inputs["hidden_states"], f32)
    Wq, bq = np.asarray(inputs["Wq"], f32), np.asarray(inputs["bq"], f32)
    Wk, bk = np.asarray(inputs["Wk"], f32), np.asarray(inputs["bk"], f32)
    Wv, bv = np.asarray(inputs["Wv"], f32), np.asarray(inputs["bv"], f32)
    Wo, bo = np.asarray(inputs["Wo"], f32), np.asarray(inputs["bo"], f32)
    ln_g, ln_b = np.asarray(inputs["ln_g"], f32), np.asarray(inputs["ln_b"], f32)
    shift = [np.asarray(inputs["shift1"], f32), np.asarray(inputs["shift2"], f32)]
    scale = [np.asarray(inputs["scale1"], f32), np.asarray(inputs["scale2"], f32)]
    gate = [np.asarray(inputs["gate1"], f32), np.asarray(inputs["gate2"], f32)]

    cos2, sin2 = _rope_tables()
    perm = _perm_matrix()
    sel2 = np.zeros((2, 2, 128), np.float32)
    sel2[0, 0, :] = 1.0
    sel2[1, 1, :] = 1.0

    qscale = HD ** -0.5
    maps = []
    for c in range(NCORES):
        b, g = c // GPC, c % GPC
        cols = slice(g * OC, (g + 1) * OC)      # head cols for QKV; out cols for Wo
        m = {"hid": np.ascontiguousarray(hs[b]),
             "cos2": cos2, "sin2": sin2, "perm": perm, "sel2": sel2}

        # effective weights per half: x_mod = xn*A_i + C_i with
        # A_i = ln_g*(1+scale_i), C_i = ln_b*(1+scale_i)+shift_i   (per batch b)
        bqk = np.zeros((128, 8), f32)
        bvv = np.zeros((2, OC), f32)
        for hf in range(2):
            A = ln_g * (1.0 + scale[hf][b])          # [H]
            C = ln_b * (1.0 + scale[hf][b]) + shift[hf][b]
            for pi, (W, bias, pref) in enumerate(
                    ((Wq, bq, "wq"), (Wk, bk, "wk"), (Wv, bv, "wv"))):
                Wc = W[cols, :]                       # [256, H]
                Weff = (Wc * A[None, :]).T            # [H, 256]
                beff = Wc @ C + bias[cols]            # [256]
                if pref == "wq":
                    Weff = Weff * qscale
                    beff = beff * qscale
                m[f"{pref}{hf}"] = np.ascontiguousarray(Weff).astype(ml_dtypes.bfloat16)
                if pref == "wv":
                    bvv[hf] = beff
                else:
                    for pair in range(2):
                        bqk[:, pi * 4 + hf * 2 + pair] = beff[pair * 128:(pair + 1) * 128]
        m["bqk"] = bqk
        m["bvv"] = bvv

        WoT = Wo[cols, :].T                      # [H(c), 256]
        # row order matches gathered ctxF: for h in 4 heads: for c2 in 2: ranks {2c2, 2c2+1}
        blocks = []
        for h in range(HPG):
            for c2 in range(2):
                for r2 in range(2):
                    r = 2 * c2 + r2
                    blocks.append(WoT[256 * r + 64 * h: 256 * r + 64 * h + 64])
        m["wo"] = np.ascontiguousarray(np.concatenate(blocks, 0)).astype(ml_dtypes.bfloat16)
        # epilogue: out = gate_i*h + hidden, h = ctx@Wo^T + bo
        #   => out = gate_i*h_mm + (gate_i*bo + hidden)   (precompute residual, transposed)
        resid = hs[b][:, cols].copy()
        resid[:HALF] += gate[0][b][cols] * bo[cols]
        resid[HALF:] += gate[1][b][cols] * bo[cols]
        m["residT"] = np.ascontiguousarray(resid.T)            # [OC, S]
        gcol = np.zeros((128, 4), f32)
        for oc2 in range(2):
            for hf in range(2):
                gcol[:, oc2 * 2 + hf] = gate[hf][b][cols][oc2 * 128:(oc2 + 1) * 128]
        m["gcol"] = gcol
        maps.append(m)
    return maps


def _build_graph(debug=False, mock_cc=False):
    nc = bacc.Bacc(None, target_bir_lowering=False, debug=False, num_devices=NCORES)

    # ---- external parameters (per-core shards) ----
    P = {}
    P["hid"] = nc.declare_dram_parameter("hid", [S, H], F32, isOutput=False)
    for hf in range(2):
        for pref in ("wq", "wk", "wv"):
            P[f"{pref}{hf}"] = nc.declare_dram_parameter(f"{pref}{hf}", [H, OC], BF16, isOutput=False)
    P["wo"] = nc.declare_dram_parameter("wo", [H, OC], BF16, isOutput=False)
    P["bqk"] = nc.declare_dram_parameter("bqk", [128, 8], F32, isOutput=False)
    P["bvv"] = nc.declare_dram_parameter("bvv", [2, OC], F32, isOutput=False)
    P["cos2"] = nc.declare_dram_parameter("cos2", [128, S], BF16, isOutput=False)
    P["sin2"] = nc.declare_dram_parameter("sin2", [128, S], BF16, isOutput=False)
    P["perm"] = nc.declare_dram_parameter("perm", [128, 128], BF16, isOutput=False)
    P["residT"] = nc.declare_dram_parameter("residT", [OC, S], F32, isOutput=False)
    P["gcol"] = nc.declare_dram_parameter("gcol", [128, 4], F32, isOutput=False)
    P["sel2"] = nc.declare_dram_parameter("sel2", [2, 2, 128], F32, isOutput=False)
    out_ext = nc.declare_dram_parameter("out", [OC, S], F32, isOutput=True)
    DBG = {}
    if debug:
        DBG["xnT"] = nc.declare_dram_parameter("d_xnT", [128, H // 128, S], BF16, isOutput=True)
        DBG["qT"] = nc.declare_dram_parameter("d_qT", [128, 2, S], BF16, isOutput=True)
        DBG["kT"] = nc.declare_dram_parameter("d_kT", [128, 2, S], BF16, isOutput=True)
        DBG["vaug"] = nc.declare_dram_parameter("d_vaug", [128, S // 128, NH // 4, HD + 2], BF16, isOutput=True)
        DBG["ctxU"] = nc.declare_dram_parameter("d_ctxU", [128, 2, S], BF16, isOutput=True)
        DBG["qb"] = nc.declare_dram_parameter("d_qb", [128, 512], BF16, isOutput=True)
        DBG["prot"] = nc.declare_dram_parameter("d_prot", [128, 512], F32, isOutput=True)
        DBG["t1"] = nc.declare_dram_parameter("d_t1", [128, 512], BF16, isOutput=True)
        DBG["t2"] = nc.declare_dram_parameter("d_t2", [128, 512], BF16, isOutput=True)
        DBG["bqk2"] = nc.declare_dram_parameter("d_bqk2", [128, 8], F32, isOutput=True)
        DBG["bvbc"] = nc.declare_dram_parameter("d_bvbc", [128, 2, OC], F32, isOutput=True)
        DBG["den"] = nc.declare_dram_parameter("d_den", [2, 1024], F32, isOutput=True)
        DBG["dtmp"] = nc.declare_dram_parameter("d_dtmp", [2, 1024], F32, isOutput=True)
        DBG["pb2"] = nc.declare_dram_parameter("d_pb2", [64, 512], F32, isOutput=True)

    # ---- internal DRAM for the collective ----
    cc_in = nc.dram_tensor("cc_in", [HPG, 64, S], BF16)
    cc_out = nc.dram_tensor("cc_out", [HPG, 4, 64, S], BF16)

    NSC = S // 128           # 16 s-chunks
    NHC = H // 128           # 8 h-chunks

    with tile.TileContext(nc) as tc:
        with tc.tile_pool(name="persist", bufs=1) as pp:
            # persistent SBUF tensors
            xnT = pp.tile([128, NHC, S], BF16, name="xnT")
            qT = pp.tile([128, 2, S], BF16, name="qT")
            kT = pp.tile([128, 2, S], BF16, name="kT")
            v_aug = pp.tile([128, NSC, HPG, HD + 2], BF16, name="v_aug")
            ctxU = pp.tile([128, 2, S], BF16, name="ctxU")
            cos_sb = pp.tile([128, S], BF16, name="cos_sb")
            sin_sb = pp.tile([128, S], BF16, name="sin_sb")
            perm_sb = pp.tile([128, 128], BF16, name="perm_sb")
            bqk_sb = pp.tile([128, 8], F32, name="bqk_sb")
            bv_sb = pp.tile([2, OC], F32, name="bv_sb")
            sel2_sb = pp.tile([2, 2, 128], F32, name="sel2_sb")
            e0_sb = pp.tile([2, 64], BF16, name="e0_sb")
            gcol_sb = pp.tile([128, 4], F32, name="gcol_sb")
            bv_bc = pp.tile([128, 2, OC], F32, name="bv_bc")
            wo_sb = pp.tile([128, NHC, OC], BF16, name="wo_sb")
            w_sb = {}
            for hf in range(2):
                for pref in ("wq", "wk", "wv"):
                    w_sb[(pref, hf)] = pp.tile([128, NHC, OC], BF16, name=f"{pref}{hf}_sb")

            nc.sync.dma_start(out=cos_sb[:], in_=P["cos2"][:])
            nc.sync.dma_start(out=sin_sb[:], in_=P["sin2"][:])
            nc.sync.dma_start(out=perm_sb[:], in_=P["perm"][:])
            nc.sync.dma_start(out=bqk_sb[:], in_=P["bqk"][:])
            nc.sync.dma_start(out=bv_sb[:], in_=P["bvv"][:])
            nc.sync.dma_start(out=gcol_sb[:], in_=P["gcol"][:])
            nc.sync.dma_start(out=sel2_sb[:], in_=P["sel2"][:])
            for hf in range(2):
                for pref in ("wq", "wk", "wv"):
                    nc.gpsimd.dma_start(
                        out=w_sb[(pref, hf)][:],
                        in_=P[f"{pref}{hf}"].rearrange("(c p) o -> p c o", p=128))
            nc.gpsimd.dma_start(out=wo_sb[:], in_=P["wo"].rearrange("(c p) o -> p c o", p=128))
            nc.gpsimd.memset(e0_sb[:], 0.0)
            nc.gpsimd.memset(e0_sb[0:1, :], 1.0)
            nc.gpsimd.memset(v_aug[:, :, :, HD:HD + 2], 1.0)

            # gate broadcast [128, OC] per half via K=1 matmul
            with tc.tile_pool(name="ps_misc", bufs=2, space="PSUM") as ps_misc:
                for hf in range(2):
                    pb = ps_misc.tile([128, OC], F32, name="pb", tag="pg")
                    nc.tensor.matmul(pb[:], sel2_sb[:, hf, :], bv_sb[:],
                                     start=True, stop=True)
                    nc.vector.tensor_copy(bv_bc[:, hf, :], pb[:])

            if True:
                # ---- Phase A: LayerNorm -> xn (bf16) -> transpose to xnT ----
                with tc.tile_pool(name="lnp", bufs=5) as lnp, \
                     tc.tile_pool(name="stat", bufs=10) as stp:
                    for sc in range(NSC):
                        hidt = lnp.tile([128, H], F32, name="hidt")
                        for _hh in range(2):
                            nc.sync.dma_start(
                                out=hidt[:, _hh * 512:(_hh + 1) * 512],
                                in_=P["hid"][sc * 128:(sc + 1) * 128, _hh * 512:(_hh + 1) * 512])
                        st6 = stp.tile([128, 2, 6], F32, name="st6", tag="st6")
                        for a in range(2):
                            nc.vector.bn_stats(st6[:, a, :], hidt[:, a * 512:(a + 1) * 512])
                        mv = stp.tile([128, 2], F32, name="mv", tag="mv")
                        nc.vector.bn_aggr(mv[:], st6[:])
                        var = stp.tile([128, 1], F32, name="var", tag="st")
                        rstd = stp.tile([128, 1], F32, name="rstd", tag="st")
                        nc.vector.tensor_scalar_add(var[:], mv[:, 1:2], EPS)
                        nc.vector.reciprocal(rstd[:], var[:])
                        nc.scalar.activation(rstd[:], rstd[:], mybir.ActivationFunctionType.Sqrt)
                        xn = lnp.tile([128, H], BF16, name="xn", tag="xn")
                        nc.gpsimd.tensor_scalar(xn[:], hidt[:], mv[:, 0:1], rstd[:],
                                                mybir.AluOpType.subtract, mybir.AluOpType.mult)
                        for hc in range(NHC):
                            nc.sync.dma_start_transpose(
                                xnT[:, hc, sc * 128:(sc + 1) * 128],
                                xn[:, hc * 128:(hc + 1) * 128])

                # ---- Phase B: QKV projections + RoPE ----
                with tc.tile_pool(name="ps_qk", bufs=3, space="PSUM") as ps_qk, \
                     tc.tile_pool(name="ps_rot", bufs=2, space="PSUM") as ps_rot, \
                     tc.tile_pool(name="ps_v", bufs=3, space="PSUM") as ps_v, \
                     tc.tile_pool(name="ropep", bufs=11) as rp:
                    # v projection: [s-chunk, o] with bias broadcast
                    for sc in range(NSC):
                        hf = 0 if sc < NSC // 2 else 1
                        pv = ps_v.tile([128, OC], F32, name="pv")
                        for hc in range(NHC):
                            nc.tensor.matmul(pv[:], xnT[:, hc, sc * 128:(sc + 1) * 128],
                                             w_sb[("wv", hf)][:, hc, :],
                                             start=(hc == 0), stop=(hc == NHC - 1))
                        nc.vector.tensor_tensor(
                            v_aug[:, sc, :, 0:HD],
                            pv[:].rearrange("p (h d) -> p h d", h=HPG),
                            bv_bc[:, hf, :].rearrange("p (h d) -> p h d", h=HPG),
                            mybir.AluOpType.add)

                    for pi, (pref, dstT) in enumerate((("wq", qT), ("wk", kT))):
                        for hf in range(2):
                            for pair in range(2):
                                for w in range(2):
                                    off = hf * HALF + w * 512
                                    pq = ps_qk.tile([128, 512], F32, name="pq")
                                    for hc in range(NHC):
                                        nc.tensor.matmul(
                                            pq[:],
                                            w_sb[(pref, hf)][:, hc, pair * 128:(pair + 1) * 128],
                                            xnT[:, hc, off:off + 512],
                                            start=(hc == 0), stop=(hc == NHC - 1))
                                    q_b = rp.tile([128, 512], BF16, name="q_b", tag="q_b")
                                    nc.vector.tensor_scalar_add(
                                        q_b[:], pq[:], bqk_sb[:, pi * 4 + hf * 2 + pair:pi * 4 + hf * 2 + pair + 1])
                                    prot = ps_rot.tile([128, 512], F32, name="prot")
                                    nc.tensor.matmul(prot[:], perm_sb[:], q_b[:],
                                                     start=True, stop=True)
                                    t1 = rp.tile([128, 512], BF16, name="t1", tag="t1")
                                    nc.gpsimd.tensor_mul(t1[:], q_b[:], cos_sb[:, off:off + 512])
                                    t2 = rp.tile([128, 512], BF16, name="t2", tag="t2")
                                    nc.vector.tensor_tensor(t2[:], prot[:], sin_sb[:, off:off + 512],
                                                            mybir.AluOpType.mult)
                                    nc.gpsimd.tensor_add(dstT[:, pair, off:off + 512],
                                                         t1[:], t2[:])
                                    if debug and pi == 0 and hf == 0 and pair == 0 and w == 0:
                                        nc.sync.dma_start(out=DBG["qb"][:], in_=q_b[:])
                                        vtmp = rp.tile([128, 512], F32, name="vtmp", tag="vtmp")
                                        nc.vector.tensor_copy(vtmp[:], prot[:])
                                        nc.sync.dma_start(out=DBG["prot"][:], in_=vtmp[:])
                                        nc.sync.dma_start(out=DBG["t1"][:], in_=t1[:])
                                        nc.sync.dma_start(out=DBG["t2"][:], in_=t2[:])
                # ---- Phase C: attention, head-pair interleaved (row-group packed) ----
                NKT = NSC  # 16 k-tiles
                with tc.tile_pool(name="ps_att", bufs=1, space="PSUM") as ps_att, \
                     tc.tile_pool(name="expp", bufs=14) as ep, \
                     tc.tile_pool(name="denp", bufs=3) as dp:
                    for pidx in range(2):            # head pair (2*pidx, 2*pidx+1)
                        for qw in range(2):
                            qoff = qw * 1024
                            pcs = [ps_att.tile([HD + 2, 1024], F32, name=f"pc{ab}",
                                               tag=f"pc{ab}") for ab in range(2)]
                            exs = {}
                            for kt in range(NKT):
                                for ab in range(2):
                                    prow = ab * 64
                                    psn = ps_att.tile([128, 1024], F32, name=f"psn{ab}",
                                                      tag=f"psn{ab}")
                                    for w2 in range(2):
                                        nc.tensor.matmul(
                                            psn[:, w2 * 512:(w2 + 1) * 512],
                                            kT[prow:prow + 64, pidx, kt * 128:(kt + 1) * 128],
                                            qT[prow:prow + 64, pidx, qoff + w2 * 512:qoff + (w2 + 1) * 512],
                                            start=True, stop=True)
                                    ex = ep.tile([128, 1024], BF16, name=f"ex{ab}", tag=f"ex{ab}")
                                    nc.scalar.activation(ex[:], psn[:], mybir.ActivationFunctionType.Exp)
                                    exs[ab] = ex
                                for ab in range(2):
                                    h = 2 * pidx + ab
                                    for w2 in range(2):
                                        nc.tensor.matmul(
                                            pcs[ab][:, w2 * 512:(w2 + 1) * 512],
                                            v_aug[:, kt, h, :],
                                            exs[ab][:, w2 * 512:(w2 + 1) * 512],
                                            start=(kt == 0), stop=(kt == NKT - 1))
                            for ab in range(2):
                                h = 2 * pidx + ab
                                prow = ab * 64
                                pc = pcs[ab]
                                # softmax denominators -> reciprocal -> broadcast -> scale
                                dsb = dp.tile([2, 1024], F32, name="dsb", tag="dsb")
                                nc.vector.tensor_copy(dsb[:], pc[HD:HD + 2, :])
                                dtmpb = dp.tile([2, 1024], BF16, name="dtmpb", tag="dtmpb")
                                nc.vector._custom_dve(
                                    __import__("concourse.dve_ops", fromlist=["RECIPROCAL_APPROX_FAST"]).RECIPROCAL_APPROX_FAST,
                                    out=dtmpb[:], in0=dsb[:],
                                    s0=-0.23549792, s1=2.0017324, imm2=2.0)
                                nc.vector.tensor_copy(ctxU[prow:prow + 64, pidx, qoff:qoff + 1024],
                                                      pc[0:HD, :])
                                for w2 in range(2):
                                    off = qoff + w2 * 512
                                    pb2 = ps_att.tile([64, 512], F32, name=f"pb2{ab}", tag=f"pc{ab}")
                                    nc.tensor.matmul(pb2[:], e0_sb[:],
                                                     dtmpb[:, w2 * 512:(w2 + 1) * 512],
                                                     start=True, stop=True)
                                    nc.vector.tensor_tensor(
                                        ctxU[prow:prow + 64, pidx, off:off + 512],
                                        ctxU[prow:prow + 64, pidx, off:off + 512],
                                        pb2[:], mybir.AluOpType.mult)
                                nc.sync.dma_start(
                                    out=cc_in[h][:, qoff:qoff + 1024],
                                    in_=ctxU[prow:prow + 64, pidx, qoff:qoff + 1024])
                                if qw == 1:
                                    # head h fully normalized -> AllGather its ctxT rows
                                    if mock_cc:
                                        # sim-only: approximate AG latency (~one shard copy)
                                        nc.gpsimd.dma_start(out=cc_out[h][0], in_=cc_in[h][:])
                                    else:
                                        nc.gpsimd.collective_compute(
                                            "AllGather", mybir.AluOpType.bypass,
                                            replica_groups=[[0, 1, 2, 3], [4, 5, 6, 7]],
                                            ins=[cc_in[h]], outs=[cc_out[h]])

            if debug:
                nc.sync.dma_start(out=DBG["bqk2"][:], in_=bqk_sb[:])
                nc.sync.dma_start(out=DBG["bvbc"][:], in_=bv_bc[:])
                nc.sync.dma_start(out=DBG["xnT"][:], in_=xnT[:])
                nc.sync.dma_start(out=DBG["qT"][:], in_=qT[:])
                nc.sync.dma_start(out=DBG["kT"][:], in_=kT[:])
                nc.sync.dma_start(out=DBG["vaug"][:], in_=v_aug[:])
                nc.sync.dma_start(out=DBG["ctxU"][:], in_=ctxU[:])

            # ---- Phase D: output projection (hT orientation) + epilogue ----
            with tc.tile_pool(name="ctxf", bufs=1) as cfp, \
                 tc.tile_pool(name="ps_o", bufs=8, space="PSUM") as ps_o, \
                 tc.tile_pool(name="outp", bufs=6) as op:
                ctxF = [cfp.tile([128, 2, S], BF16, name=f"ctxF{j}") for j in range(HPG)]
                for j in range(HPG):
                    for _c2 in range(2):
                        nc.gpsimd.dma_start(
                            out=ctxF[j][:, _c2, :],
                            in_=cc_out[j][2 * _c2:2 * _c2 + 2].rearrange("b p s -> (b p) s"))
                residT_sb = cfp.tile([128, 2, S], F32, name="residT_sb")
                for _a in range(2):
                    for _hw in range(2):
                        nc.gpsimd.dma_start(
                            out=residT_sb[:, _a, _hw * HALF:(_hw + 1) * HALF],
                            in_=P["residT"][_a * 128:(_a + 1) * 128, _hw * HALF:(_hw + 1) * HALF]
                                .rearrange("(a p) s -> p (a s)", p=128))
                for oc2 in range(2):                 # 128-wide output-column chunks
                    for w in range(4):               # 512-wide s windows
                        hf = 0 if w < 2 else 1
                        po = ps_o.tile([128, 512], F32, name="po")
                        for idx in range(8):
                            j, c2 = idx // 2, idx % 2
                            nc.tensor.matmul(
                                po[:], wo_sb[:, idx, oc2 * 128:(oc2 + 1) * 128],
                                ctxF[j][:, c2, w * 512:(w + 1) * 512],
                                start=(idx == 0), stop=(idx == 7))
                        t = op.tile([128, 512], F32, name="t", tag="t")
                        nc.vector.tensor_scalar_mul(t[:], po[:], gcol_sb[:, oc2 * 2 + hf:oc2 * 2 + hf + 1])
                        ot = op.tile([128, 512], F32, name="ot", tag="ot")
                        nc.gpsimd.tensor_add(ot[:], t[:],
                                             residT_sb[:, oc2, w * 512:(w + 1) * 512])
                        _oeng = nc.sync if (oc2 * 4 + w) % 2 == 0 else nc.gpsimd
                        _oeng.dma_start(
                            out=out_ext[oc2 * 128:(oc2 + 1) * 128, w * 512:(w + 1) * 512],
                            in_=ot[:])

    nc.compile()
    return nc


def kernel(**inputs) -> np.ndarray:
    if "nc" not in _CACHE:
        _CACHE["nc"] = _build_graph()
    nc = _CACHE["nc"]
    in_maps = _build_core_inputs(inputs)
    res = run_bass_kernel_spmd(nc, in_maps, core_ids=list(range(NCORES))).results
    out = np.empty((B, S, H), np.float32)
    for c in range(NCORES):
        b, g = c // GPC, c % GPC
        out[b, :, g * OC:(g + 1) * OC] = res[c]["out"].T
    return out

